# revision 11
# baseline (speedup 1.0000x reference)
"""Trainium2 Bass kernel for nn_Attention_73770358276185.

Per-batch computation (B=8, one batch per NeuronCore, data-parallel):
    f = gelu(BN(Wf @ q + bf))            [64, 4096]
    g = gelu(BN(Wg @ k + bg))            [64, 4096]
    h = gelu(BN(Wh @ k + bh))            [256, 4096]
    s[i,j] = sum_l g[l,i] f[l,j]         [4096, 4096]
    beta = softmax_j(s)
    o[i,c] = sum_j beta[i,j] h[c,j]
    out = gamma * o.T + q

v2 design notes:
  - The whole attention phase runs the PE in 64x128 row-tiled mode
    (tile_position (0,0)/(64,0)): mm1 (contraction over l=64) computes two
    j-blocks concurrently; mm2 splits its K=128 contraction into two
    64-row halves (oA/oB partial sums, merged on DVE at the epilogue).
  - f and g are replicated into both partition halves for free by
    duplicating the projection weights to M=128 columns.
  - softmax needs no max-subtraction (s_max ~ 69 < fp32 exp range); the
    row sum rides the mm2 stream as a ones-column in h_aug (258-wide rhs).
  - The residual q is served from the tf32-rounded q tiles (error
    q*2^-12, far below tolerance), halving the q DMA traffic.
  - exp is done in [128, 1024] tiles from PSUM; all gelus strictly
    precede all exps so the ACT table set switches exactly once.
"""
import sys

for _p in ("/opt/trn_rl_repo", "/root/.axon_site/_ro/trn_rl_repo"):
    if _p not in sys.path:
        sys.path.insert(0, _p)

import numpy as np

import concourse.bacc as bacc
import concourse.tile as tile
import concourse.mybir as mybir
from concourse.bass_utils import run_bass_kernel_spmd

P = 128
B = 8
N = 4096          # sequence positions
C1 = 256          # dim1 (q channels / h channels)
C2 = 128          # dim2 (k channels)
L = 64            # layer = dim1 // 4 (f/g channels)
EPS = 1e-5

NJB = N // P      # 32 j-blocks
IC = 256          # i-chunk (2 ib blocks of 128)
NIC = N // IC     # 16 i-chunks
JG = 4            # j-blocks per group
NGRP = NJB // JG  # 8 groups per i-chunk
HST = 258         # h_aug row stride (256 ch + ones col + pad)
CHK = 1024        # load/projection chunk (columns)
NCHK = N // CHK   # 4

F32 = mybir.dt.float32
F32R = mybir.dt.float32r
AF = mybir.ActivationFunctionType
MUL = mybir.AluOpType.mult

TILED = True
GELU_FN = AF.Gelu  # debug: swap to AF.Relu for CoreSim numeric execution

_BUILT = None  # (nc) cache — the program is input-value independent


def _round_tf32(x):
    """Round fp32 to float32r (drop 12 mantissa bits, round-to-nearest)."""
    v = np.ascontiguousarray(x, dtype=np.float32).view(np.uint32).astype(np.uint64)
    half = np.uint64(0x7FF)
    lsb = (v >> np.uint64(12)) & np.uint64(1)
    v = (v + half + lsb) & np.uint64(0xFFFFF000)
    return v.astype(np.uint32).view(np.float32)


def _build(repeat=1, tiled=TILED):
    nc = bacc.Bacc("TRN2", target_bir_lowering=False, debug=False)

    k2r = nc.dram_tensor("k2r", [C2, N], F32R, kind="ExternalInput")
    q2r = nc.dram_tensor("q2r", [C1, N], F32R, kind="ExternalInput")
    wf2T = nc.dram_tensor("wf2T", [C1, P], F32R, kind="ExternalInput")
    wg2T = nc.dram_tensor("wg2T", [C2, P], F32R, kind="ExternalInput")
    whT = nc.dram_tensor("whT", [C2, C1], F32R, kind="ExternalInput")
    df2 = nc.dram_tensor("df2", [P, 1], F32, kind="ExternalInput")
    dg2 = nc.dram_tensor("dg2", [P, 1], F32, kind="ExternalInput")
    dht4 = nc.dram_tensor("dht4", [P, 4 * C1], F32, kind="ExternalInput")
    gmb = nc.dram_tensor("gmb", [P, 1], F32, kind="ExternalInput")
    ident = nc.dram_tensor("ident", [P, P], F32, kind="ExternalInput")
    ones2 = nc.dram_tensor("ones2", [P, 2 * NJB], F32R, kind="ExternalInput")
    o_out = nc.dram_tensor("o_out", [C1, N], F32, kind="ExternalOutput")

    with tile.TileContext(nc) as tc:
        with (
            tc.tile_pool(name="const", bufs=1) as cp,
            tc.tile_pool(name="ps", bufs=2, space="PSUM") as psp,
            tc.tile_pool(name="oacc", bufs=4, space="PSUM") as op,
            tc.tile_pool(name="ex", bufs=3) as exp_,
            tc.tile_pool(name="ep", bufs=4) as ep,
            tc.tile_pool(name="outst", bufs=2) as outp,
        ):
            # ---- constant/weight loads (outside repeat loop) -------------
            wf2a = cp.tile([P, P], F32R, tag="wf2a")
            nc.sync.dma_start(wf2a[:], wf2T[0:P, :])
            wf2b = cp.tile([P, P], F32R, tag="wf2b")
            nc.sync.dma_start(wf2b[:], wf2T[P:C1, :])
            wg2 = cp.tile([C2, P], F32R, tag="wg2")
            nc.sync.dma_start(wg2[:], wg2T[:, :])
            wh = cp.tile([C2, C1], F32R, tag="wh")
            nc.sync.dma_start(wh[:], whT[:, :])
            dft = cp.tile([P, 1], F32, tag="df")
            nc.sync.dma_start(dft[:], df2[:, :])
            dgt = cp.tile([P, 1], F32, tag="dg")
            nc.sync.dma_start(dgt[:], dg2[:, :])

            # h_aug holds hT per j-block ([j, c] layout) + ones/pad columns
            h_aug = cp.tile([P, NJB * HST], F32R, tag="h")
            h3 = h_aug.rearrange("p (j c) -> p j c", c=HST)

            # interleaved chunk loads: k chunk c, then q chunk c (both
            # halves); the non-critical constants (dht/ones/gm/idt) are
            # slotted between chunks so they don't delay the first
            # projections.
            k_sb = cp.tile([C2, N], F32R, tag="k")
            q_sb = [cp.tile([P, N], F32R, tag=f"q{cb}", name=f"q{cb}")
                    for cb in range(2)]
            dht = cp.tile([P, 4 * C1], F32, tag="dh")
            gm = cp.tile([P, 1], F32, tag="gm")
            idt = cp.tile([P, P], F32, tag="id")
            for c in range(NCHK):
                sl = slice(c * CHK, (c + 1) * CHK)
                nc.sync.dma_start(k_sb[:, sl], k2r[:, sl])
                nc.sync.dma_start(q_sb[0][:, sl], q2r[0:P, sl])
                nc.sync.dma_start(q_sb[1][:, sl], q2r[P:C1, sl])
                if c == 0:
                    nc.sync.dma_start(dht[:], dht4[:, :])
                    nc.sync.dma_start(
                        h3[:, :, C1:HST],
                        ones2[:].rearrange("p (j c) -> p j c", c=2),
                    )
                elif c == 1:
                    nc.sync.dma_start(gm[:], gmb[:, :])
                    nc.sync.dma_start(idt[:], ident[:, :])

            f2 = cp.tile([P, N], F32R, tag="f2")
            g2 = cp.tile([P, N], F32R, tag="g2")

            env = dict(
                nc=nc, psp=psp, op=op, exp_=exp_, ep=ep, outp=outp,
                wf2a=wf2a, wf2b=wf2b, wg2=wg2, wh=wh, dft=dft, dgt=dgt,
                dht=dht, gm=gm, idt=idt, h_aug=h_aug, h3=h3, k_sb=k_sb,
                q_sb=q_sb, f2=f2, g2=g2, o_out=o_out, tiled=tiled,
            )

            import contextlib
            loop_cm = tc.For_i(0, repeat, 1) if repeat > 1 else contextlib.nullcontext()
            with loop_cm:
                _emit_body(env)

    nc.finalize()
    return nc


def _emit_body(env):
    nc = env["nc"]
    psp = env["psp"]; op = env["op"]; exp_ = env["exp_"]
    ep = env["ep"]; outp = env["outp"]
    wf2a = env["wf2a"]; wf2b = env["wf2b"]; wg2 = env["wg2"]; wh = env["wh"]
    dft = env["dft"]; dgt = env["dgt"]; dht = env["dht"]; gm = env["gm"]
    idt = env["idt"]; h_aug = env["h_aug"]; h3 = env["h3"]; k_sb = env["k_sb"]
    q_sb = env["q_sb"]; f2 = env["f2"]; g2 = env["g2"]; o_out = env["o_out"]
    tiled = env["tiled"]

    # ---- PE warm-up: dummy matmuls on the first-loaded weight tile keep
    # the HAM activity window busy while the k/q chunk DMAs stream in, so
    # the real projections start at the full 2.4 GHz clock.
    dmy = op.tile([P, P], F32, tag="oacc", name="dmy")
    for _ in range(12):
        nc.tensor.matmul(dmy[:], wf2a[:], wf2a[:], start=True, stop=True)

    # ---- phase A: projections (PE in full 128-row mode) ------------------
    for c in range(NCHK):
        sl = slice(c * CHK, (c + 1) * CHK)
        gps = psp.tile([P, CHK], F32, tag="sT", name="gps")
        for hh in range(2):
            ssl = slice(c * CHK + hh * 512, c * CHK + (hh + 1) * 512)
            nc.tensor.matmul(gps[:, hh * 512:(hh + 1) * 512], wg2[:],
                             k_sb[:, ssl], start=True, stop=True)
        nc.scalar.activation(g2[:, sl], gps[:], GELU_FN, bias=dgt[:])

        for hh in range(2):
            hps = psp.tile([P, CHK], F32, tag="sT", name="hps")
            jb0 = 8 * c + 4 * hh
            for jbl in range(4):
                jb = jb0 + jbl
                nc.tensor.matmul(hps[:, jbl * C1:(jbl + 1) * C1],
                                 k_sb[:, jb * P:(jb + 1) * P], wh[:],
                                 start=True, stop=True)
            nc.vector.tensor_add(hps[:], hps[:], dht[:])
            nc.scalar.activation(h3[:, jb0:jb0 + 4, 0:C1], hps[:], GELU_FN)

    for c in range(NCHK):
        sl = slice(c * CHK, (c + 1) * CHK)
        fps = psp.tile([P, CHK], F32, tag="sT", name="fps")
        for hh in range(2):
            osl = slice(hh * 512, (hh + 1) * 512)
            ssl = slice(c * CHK + hh * 512, c * CHK + (hh + 1) * 512)
            nc.tensor.matmul(fps[:, osl], wf2a[:], q_sb[0][:, ssl],
                             start=True, stop=False)
            nc.tensor.matmul(fps[:, osl], wf2b[:], q_sb[1][:, ssl],
                             start=False, stop=True)
        nc.scalar.activation(f2[:, sl], fps[:], GELU_FN, bias=dft[:])

    # ---- attention main loop (software-pipelined emission) ---------------
    state = {}

    def emit_mm1(ic, grp):
        sT = psp.tile([P, JG * IC], F32, tag="sT", name="sT")
        icsl = slice(ic * IC, (ic + 1) * IC)
        if tiled:
            # pairs (0,2) and (1,3) run concurrently on row-tiles T0/T8;
            # each pair writes two different PSUM banks.
            for jbl0 in (0, 1):
                for t, jbl in enumerate((jbl0, jbl0 + 2)):
                    jb = grp * JG + jbl
                    half = slice(t * L, (t + 1) * L)
                    nc.tensor.matmul(
                        sT[:, jbl * IC:(jbl + 1) * IC],
                        f2[half, jb * P:(jb + 1) * P],
                        g2[half, icsl],
                        start=True, stop=True,
                        tile_position=(t * L, 0))
        else:
            for jbl in range(JG):
                jb = grp * JG + jbl
                nc.tensor.matmul(sT[:, jbl * IC:(jbl + 1) * IC],
                                 f2[:, jb * P:(jb + 1) * P], g2[:, icsl],
                                 start=True, stop=True)
        ex = exp_.tile([P, JG * IC], F32R, tag="ex", name="ex")
        nc.scalar.activation(ex[:], sT[:], AF.Exp)
        return ex

    def emit_mm2(ic, grp, ex):
        if grp == 0:
            state[ic] = dict(
                oA=[op.tile([P, HST], F32, tag="oacc", name=f"oA{ib}")
                    for ib in range(2)],
                oB=[op.tile([P, HST], F32, tag="oacc", name=f"oB{ib}")
                    for ib in range(2)] if tiled else None,
            )
        oA = state[ic]["oA"]; oB = state[ic]["oB"]
        for jbl in range(JG):
            jb = grp * JG + jbl
            hsl = slice(jb * HST, (jb + 1) * HST)
            st = jb == 0
            sp = jb == NJB - 1
            for ib in range(2):
                exsl = slice(jbl * IC + ib * P, jbl * IC + (ib + 1) * P)
                if tiled:
                    nc.tensor.matmul(oA[ib][:], ex[0:L, exsl],
                                     h_aug[0:L, hsl],
                                     start=st, stop=sp, tile_position=(0, 0))
                    nc.tensor.matmul(oB[ib][:], ex[L:P, exsl],
                                     h_aug[L:P, hsl],
                                     start=st, stop=sp, tile_position=(L, 0))
                else:
                    nc.tensor.matmul(oA[ib][:], ex[:, exsl], h_aug[:, hsl],
                                     start=st, stop=sp)

    def emit_epilogue(ic):
        oA = state[ic]["oA"]; oB = state[ic]["oB"]
        oscs = []
        for ib in range(2):
            if tiled:
                # DVE may read at most one PSUM operand per instruction:
                # stage oB through SBUF, then add oA (PSUM) + staged (SBUF).
                obs = ep.tile([P, HST], F32, tag="obs", name="obs")
                nc.vector.tensor_copy(obs[:], oB[ib][:])
                om = ep.tile([P, HST], F32, tag="om", name="om")
                nc.vector.tensor_add(om[:], oA[ib][:], obs[:])
                rs = om[:, C1:C1 + 1]
                src = om[:, 0:C1]
            else:
                rs = oA[ib][:, C1:C1 + 1]
                src = oA[ib][:, 0:C1]
            rv = ep.tile([P, 1], F32, tag="rv", name="rv")
            nc.vector.reciprocal(rv[:], rs[:])
            osc = ep.tile([P, C1], F32, tag="osc", name="osc")
            nc.vector.tensor_scalar(osc[:], src[:], rv[:], gm[:],
                                    op0=MUL, op1=MUL)
            oscs.append(osc)
        ost = [outp.tile([P, IC], F32, tag=f"ost{cb}", name=f"ost{cb}")
               for cb in range(2)]
        for ib in range(2):
            for cb in range(2):
                oT = op.tile([P, P], F32, tag="oacc", name="oT")
                nc.tensor.transpose(oT[:], oscs[ib][:, cb * P:(cb + 1) * P],
                                    idt[:])
                nc.vector.tensor_add(
                    ost[cb][:, ib * P:(ib + 1) * P], oT[:],
                    q_sb[cb][:, ic * IC + ib * P:ic * IC + (ib + 1) * P])
        for cb in range(2):
            nc.sync.dma_start(o_out[cb * P:(cb + 1) * P, ic * IC:(ic + 1) * IC],
                              ost[cb][:])
        del state[ic]

    # Pipelined emission: mm1 runs one group ahead of mm2; the epilogue
    # (which puts transposes on the PE) is delayed one further group so the
    # exp stream never starves at an i-chunk boundary.
    groups = [(ic, grp) for ic in range(NIC) for grp in range(NGRP)]
    pending = None
    epi_due = None
    for (ic, grp) in groups:
        ex = emit_mm1(ic, grp)
        if epi_due is not None:
            emit_epilogue(epi_due)
            epi_due = None
        if pending is not None:
            pic, pgrp, pex = pending
            emit_mm2(pic, pgrp, pex)
            if pgrp == NGRP - 1:
                epi_due = pic
        pending = (ic, grp, ex)
    if epi_due is not None:
        emit_epilogue(epi_due)
    pic, pgrp, pex = pending
    emit_mm2(pic, pgrp, pex)
    emit_epilogue(pic)


def _preprocess(inputs, tiled=TILED):
    """Fold conv bias + BN into effective weights/biases, per-core input maps."""
    f32 = np.float32
    q = np.ascontiguousarray(inputs["q"], dtype=f32)[..., 0]   # [B, 256, N]
    k = np.ascontiguousarray(inputs["k"], dtype=f32)[..., 0]   # [B, 128, N]

    def fold(W, b, scale, bias, mean, var):
        inv = (np.asarray(scale, f32) /
               np.sqrt(np.asarray(var, f32) + f32(EPS))).astype(f32)
        W_eff = (inv[:, None] * np.asarray(W, f32)).astype(f32)
        delta = ((np.asarray(b, f32) - np.asarray(mean, f32)) * inv
                 + np.asarray(bias, f32)).astype(f32)
        return W_eff, delta

    Wf_e, d_f = fold(inputs["Wf"], inputs["bf"], inputs["fs"], inputs["fb"],
                     inputs["fm"], inputs["fv"])
    Wg_e, d_g = fold(inputs["Wg"], inputs["bg"], inputs["gs"], inputs["gb"],
                     inputs["gm"], inputs["gv"])
    Wh_e, d_h = fold(inputs["Wh"], inputs["bh"], inputs["hs"], inputs["hb"],
                     inputs["hm"], inputs["hv"])

    gamma = f32(np.asarray(inputs["gamma"], f32).reshape(-1)[0])
    if tiled:
        wf2 = np.concatenate([Wf_e.T, Wf_e.T], axis=1)        # [256, 128]
        wg2 = np.concatenate([Wg_e.T, Wg_e.T], axis=1)        # [128, 128]
        df2 = np.concatenate([d_f, d_f]).reshape(P, 1)
        dg2 = np.concatenate([d_g, d_g]).reshape(P, 1)
    else:
        # upper half must be exactly zero after gelu: zero weights and a
        # large negative bias (gelu(-30) underflows to 0)
        wf2 = np.concatenate([Wf_e.T, np.zeros_like(Wf_e.T)], axis=1)
        wg2 = np.concatenate([Wg_e.T, np.zeros_like(Wg_e.T)], axis=1)
        df2 = np.concatenate([d_f, np.full(L, -30.0, f32)]).reshape(P, 1)
        dg2 = np.concatenate([d_g, np.full(L, -30.0, f32)]).reshape(P, 1)

    ones2 = np.zeros((P, 2 * NJB), f32)
    ones2[:, 0::2] = 1.0

    shared = {
        "wf2T": _round_tf32(wf2),
        "wg2T": _round_tf32(wg2),
        "whT": _round_tf32(Wh_e.T),                           # [128, 256]
        "df2": df2,
        "dg2": dg2,
        "dht4": np.tile(np.broadcast_to(d_h, (P, C1)), (1, 4)).copy(),
        "gmb": np.full((P, 1), gamma, f32),
        "ident": np.eye(P, dtype=f32),
        "ones2": ones2,
    }
    in_maps = []
    for b_ in range(B):
        m = dict(shared)
        m["q2r"] = _round_tf32(q[b_])
        m["k2r"] = _round_tf32(k[b_])
        in_maps.append(m)
    return in_maps


def _get_nc():
    global _BUILT
    if _BUILT is None:
        _BUILT = _build()
    return _BUILT


def kernel(**inputs):
    nc = _get_nc()
    in_maps = _preprocess(inputs)
    res = run_bass_kernel_spmd(nc, in_maps, core_ids=list(range(B)))
    out = np.stack([res.results[i]["o_out"] for i in range(B)])
    return out[..., None].astype(np.float32)


if __name__ == "__main__":
    rng = np.random.default_rng(0)
    fake = {
        "q": rng.standard_normal((B, C1, N, 1), dtype=np.float32),
        "k": rng.standard_normal((B, C2, N, 1), dtype=np.float32),
        "Wf": rng.standard_normal((L, C1), dtype=np.float32) * 0.06,
        "bf": rng.standard_normal(L, dtype=np.float32) * 0.01,
        "fs": rng.random(L, dtype=np.float32) + 0.5,
        "fb": rng.standard_normal(L, dtype=np.float32) * 0.1,
        "fm": rng.standard_normal(L, dtype=np.float32) * 0.1,
        "fv": rng.random(L, dtype=np.float32) + 0.5,
        "Wg": rng.standard_normal((L, C2), dtype=np.float32) * 0.09,
        "bg": rng.standard_normal(L, dtype=np.float32) * 0.01,
        "gs": rng.random(L, dtype=np.float32) + 0.5,
        "gb": rng.standard_normal(L, dtype=np.float32) * 0.1,
        "gm": rng.standard_normal(L, dtype=np.float32) * 0.1,
        "gv": rng.random(L, dtype=np.float32) + 0.5,
        "Wh": rng.standard_normal((C1, C2), dtype=np.float32) * 0.09,
        "bh": rng.standard_normal(C1, dtype=np.float32) * 0.01,
        "hs": rng.random(C1, dtype=np.float32) + 0.5,
        "hb": rng.standard_normal(C1, dtype=np.float32) * 0.1,
        "hm": rng.standard_normal(C1, dtype=np.float32) * 0.1,
        "hv": rng.random(C1, dtype=np.float32) + 0.5,
        "gamma": np.array([-1.1], dtype=np.float32),
    }
    out = kernel(**fake)
    print("out", out.shape, out.dtype, float(np.abs(out).max()))


# revision 22
# speedup vs baseline: 1.0378x; 1.0378x over previous
"""Trainium2 Bass kernel for nn_Attention_73770358276185.

Per-batch computation (B=8, one batch per NeuronCore, data-parallel):
    f = gelu(BN(Wf @ q + bf))            [64, 4096]
    g = gelu(BN(Wg @ k + bg))            [64, 4096]
    h = gelu(BN(Wh @ k + bh))            [256, 4096]
    s[i,j] = sum_l g[l,i] f[l,j]         [4096, 4096]
    beta = softmax_j(s)
    o[i,c] = sum_j beta[i,j] h[c,j]
    out = gamma * o.T + q

Design notes (v2):
  - sT layout [j, i]: the softmax contraction (j) lands on the matmul
    partition dim for the second matmul, so no transposes of the
    attention matrix are needed.  All big matmuls are float32r (TF32)
    with moving dims >= 512 (258 for mm2) — the fp32r full-rate
    threshold on real HW sits above 256, so N=256 shapes are slow.
  - Row-tiled (tile_position) variants were tried and are NOT faster:
    per-tile LDWEIGHTS cannot be hidden behind the same tile's matmul,
    so two K=64 tiles stream no faster than one K=128 matmul.
  - softmax needs no max-subtraction (s_max ~ 69 < fp32 exp range); the
    row sum rides the mm2 stream as a ones-column in h_aug (258-wide rhs).
  - The residual q is served from the tf32-rounded q tiles (error
    q*2^-12, far below tolerance), halving the q DMA traffic.
  - exp is done in [128, 1024] tiles from PSUM; all gelus strictly
    precede all exps so the ACT table set switches exactly once
    (gelu and exp live in different ACT table sets).
  - Startup: k/q are loaded in interleaved 1KB chunks with projections
    consuming them as they land; dummy matmuls keep the PE HAM window
    warm during the first loads; the epilogue is delayed one group so
    the exp stream isn't starved at i-chunk boundaries.
"""
import sys

for _p in ("/opt/trn_rl_repo", "/root/.axon_site/_ro/trn_rl_repo"):
    if _p not in sys.path:
        sys.path.insert(0, _p)

import numpy as np

import concourse.bacc as bacc
import concourse.tile as tile
import concourse.mybir as mybir
from concourse.bass_utils import run_bass_kernel_spmd

P = 128
B = 8
N = 4096          # sequence positions
C1 = 256          # dim1 (q channels / h channels)
C2 = 128          # dim2 (k channels)
L = 64            # layer = dim1 // 4 (f/g channels)
EPS = 1e-5

NJB = N // P      # 32 j-blocks
IC = 256          # i-chunk (2 ib blocks of 128)
NIC = N // IC     # 16 i-chunks
JG = 4            # j-blocks per group
NGRP = NJB // JG  # 8 groups per i-chunk
NIB = IC // P     # 2 ib blocks
HST = 258         # h_aug row stride (256 ch + ones col + pad)
CHK = 1024        # load/projection chunk (columns)
NCHK = N // CHK   # 4

F32 = mybir.dt.float32
F32R = mybir.dt.float32r
BF16 = mybir.dt.bfloat16
AF = mybir.ActivationFunctionType
MUL = mybir.AluOpType.mult

TILED = False
GELU_FN = AF.Gelu  # debug: swap to AF.Relu for CoreSim numeric execution

_BUILT = None  # (nc) cache — the program is input-value independent


def _round_tf32(x):
    """Round fp32 to float32r (drop 12 mantissa bits, round-to-nearest)."""
    v = np.ascontiguousarray(x, dtype=np.float32).view(np.uint32).astype(np.uint64)
    half = np.uint64(0x7FF)
    lsb = (v >> np.uint64(12)) & np.uint64(1)
    v = (v + half + lsb) & np.uint64(0xFFFFF000)
    return v.astype(np.uint32).view(np.float32)


def _build(repeat=1, tiled=TILED):
    nc = bacc.Bacc("TRN2", target_bir_lowering=False, debug=False)

    k2r = nc.dram_tensor("k2r", [C2, N], F32R, kind="ExternalInput")
    q2r = nc.dram_tensor("q2r", [C1, N], F32R, kind="ExternalInput")
    wf2T = nc.dram_tensor("wf2T", [C1, P], F32R, kind="ExternalInput")
    wg2T = nc.dram_tensor("wg2T", [C2, P], F32R, kind="ExternalInput")
    whT = nc.dram_tensor("whT", [C2, C1], F32R, kind="ExternalInput")
    df2 = nc.dram_tensor("df2", [P, 1], F32, kind="ExternalInput")
    dg2 = nc.dram_tensor("dg2", [P, 1], F32, kind="ExternalInput")
    dht4 = nc.dram_tensor("dht4", [P, 4 * C1], F32, kind="ExternalInput")
    gmb = nc.dram_tensor("gmb", [P, 1], F32, kind="ExternalInput")
    ident = nc.dram_tensor("ident", [P, P], F32, kind="ExternalInput")
    ones2 = nc.dram_tensor("ones2", [P, 2 * NJB], F32R, kind="ExternalInput")
    o_out = nc.dram_tensor("o_out", [C1, N], F32, kind="ExternalOutput")

    with tile.TileContext(nc) as tc:
        with (
            tc.tile_pool(name="const", bufs=1) as cp,
            tc.tile_pool(name="ps", bufs=3, space="PSUM") as psp,
            tc.tile_pool(name="oacc", bufs=2, space="PSUM") as op,
            tc.tile_pool(name="ex", bufs=3) as exp_,
            tc.tile_pool(name="ep", bufs=4) as ep,
            tc.tile_pool(name="outst", bufs=2) as outp,
        ):
            # ---- constant/weight loads (outside repeat loop) -------------
            wf2a = cp.tile([P, P], F32R, tag="wf2a")
            nc.sync.dma_start(wf2a[:], wf2T[0:P, :])
            wf2b = cp.tile([P, P], F32R, tag="wf2b")
            nc.sync.dma_start(wf2b[:], wf2T[P:C1, :])
            wg2 = cp.tile([C2, P], F32R, tag="wg2")
            nc.sync.dma_start(wg2[:], wg2T[:, :])
            wh = cp.tile([C2, C1], F32R, tag="wh")
            nc.sync.dma_start(wh[:], whT[:, :])
            dft = cp.tile([P, 1], F32, tag="df")
            nc.sync.dma_start(dft[:], df2[:, :])
            dgt = cp.tile([P, 1], F32, tag="dg")
            nc.sync.dma_start(dgt[:], dg2[:, :])

            # h_aug holds hT per j-block ([j, c] layout) + ones/pad columns
            h_aug = cp.tile([P, NJB * HST], F32R, tag="h")
            h3 = h_aug.rearrange("p (j c) -> p j c", c=HST)

            # interleaved chunk loads: k chunk c, then q chunk c (both
            # halves); the non-critical constants (dht/ones/gm/idt) are
            # slotted between chunks so they don't delay the first
            # projections.
            k_sb = cp.tile([C2, N], F32R, tag="k")
            q_sb = [cp.tile([P, N], F32R, tag=f"q{cb}", name=f"q{cb}")
                    for cb in range(2)]
            dht = cp.tile([P, 4 * C1], F32, tag="dh")
            gm = cp.tile([P, 1], F32, tag="gm")
            idt = cp.tile([P, P], F32, tag="id")
            for c in range(NCHK):
                sl = slice(c * CHK, (c + 1) * CHK)
                nc.sync.dma_start(k_sb[:, sl], k2r[:, sl])
                nc.sync.dma_start(q_sb[0][:, sl], q2r[0:P, sl])
                nc.sync.dma_start(q_sb[1][:, sl], q2r[P:C1, sl])
                if c == 0:
                    nc.sync.dma_start(dht[:], dht4[:, :])
                    nc.sync.dma_start(
                        h3[:, :, C1:HST],
                        ones2[:].rearrange("p (j c) -> p j c", c=2),
                    )
                elif c == 1:
                    nc.sync.dma_start(gm[:], gmb[:, :])

            f2 = cp.tile([P, N], F32R, tag="f2")
            g2 = cp.tile([P, N], F32R, tag="g2")

            env = dict(
                nc=nc, psp=psp, op=op, exp_=exp_, ep=ep, outp=outp,
                wf2a=wf2a, wf2b=wf2b, wg2=wg2, wh=wh, dft=dft, dgt=dgt,
                dht=dht, gm=gm, h_aug=h_aug, h3=h3, k_sb=k_sb,
                q_sb=q_sb, f2=f2, g2=g2, o_out=o_out, tiled=tiled,
            )

            env["warmup"] = repeat == 1
            import contextlib
            loop_cm = tc.For_i(0, repeat, 1) if repeat > 1 else contextlib.nullcontext()
            with loop_cm:
                _emit_body(env)

    nc.finalize()
    return nc


def _emit_body(env):
    nc = env["nc"]
    psp = env["psp"]; op = env["op"]; exp_ = env["exp_"]
    ep = env["ep"]; outp = env["outp"]
    wf2a = env["wf2a"]; wf2b = env["wf2b"]; wg2 = env["wg2"]; wh = env["wh"]
    dft = env["dft"]; dgt = env["dgt"]; dht = env["dht"]; gm = env["gm"]
    h_aug = env["h_aug"]; h3 = env["h3"]; k_sb = env["k_sb"]
    q_sb = env["q_sb"]; f2 = env["f2"]; g2 = env["g2"]; o_out = env["o_out"]
    tiled = env["tiled"]

    # ---- PE warm-up: dummy matmuls on the first-loaded weight tile keep
    # the HAM activity window busy while the k/q chunk DMAs stream in, so
    # the real projections start at the full 2.4 GHz clock.  (Single-shot
    # builds only — inside a repeat loop the PE is already warm.)
    if env.get("warmup", True):
        dmy = op.tile([P, P], F32, tag="oacc", name="dmy")
        for _ in range(8):
            nc.tensor.matmul(dmy[:], wf2a[:], wf2a[:], start=True, stop=True)

    # ---- phase A: projections (PE in full 128-row mode) ------------------
    for c in range(NCHK):
        sl = slice(c * CHK, (c + 1) * CHK)
        gps = psp.tile([P, CHK], F32, tag="sT", name="gps")
        for hh in range(2):
            ssl = slice(c * CHK + hh * 512, c * CHK + (hh + 1) * 512)
            nc.tensor.matmul(gps[:, hh * 512:(hh + 1) * 512], wg2[:],
                             k_sb[:, ssl], start=True, stop=True)
        nc.scalar.activation(g2[:, sl], gps[:], GELU_FN, bias=dgt[:])

        for hh in range(2):
            hps = psp.tile([P, CHK], F32, tag="sT", name="hps")
            jb0 = 8 * c + 4 * hh
            for jbl in range(4):
                jb = jb0 + jbl
                nc.tensor.matmul(hps[:, jbl * C1:(jbl + 1) * C1],
                                 k_sb[:, jb * P:(jb + 1) * P], wh[:],
                                 start=True, stop=True)
            nc.vector.tensor_add(hps[:], hps[:], dht[:])
            nc.scalar.activation(h3[:, jb0:jb0 + 4, 0:C1], hps[:], GELU_FN)

    for c in range(NCHK):
        sl = slice(c * CHK, (c + 1) * CHK)
        fps = psp.tile([P, CHK], F32, tag="sT", name="fps")
        for hh in range(2):
            osl = slice(hh * 512, (hh + 1) * 512)
            ssl = slice(c * CHK + hh * 512, c * CHK + (hh + 1) * 512)
            nc.tensor.matmul(fps[:, osl], wf2a[:], q_sb[0][:, ssl],
                             start=True, stop=False)
            nc.tensor.matmul(fps[:, osl], wf2b[:], q_sb[1][:, ssl],
                             start=False, stop=True)
        nc.scalar.activation(f2[:, sl], fps[:], GELU_FN, bias=dft[:])

    # ---- attention main loop (software-pipelined emission) ---------------
    state = {}

    def emit_mm1(ic, grp):
        sT = psp.tile([P, JG * IC], F32, tag="sT", name="sT")
        icsl = slice(ic * IC, (ic + 1) * IC)
        if tiled:
            # pairs (0,2) and (1,3) run concurrently on row-tiles T0/T8;
            # each pair writes two different PSUM banks.
            for jbl0 in (0, 1):
                for t, jbl in enumerate((jbl0, jbl0 + 2)):
                    jb = grp * JG + jbl
                    half = slice(t * L, (t + 1) * L)
                    nc.tensor.matmul(
                        sT[:, jbl * IC:(jbl + 1) * IC],
                        f2[half, jb * P:(jb + 1) * P],
                        g2[half, icsl],
                        start=True, stop=True,
                        tile_position=(t * L, 0))
        else:
            for jbl in range(JG):
                jb = grp * JG + jbl
                nc.tensor.matmul(sT[:, jbl * IC:(jbl + 1) * IC],
                                 f2[:, jb * P:(jb + 1) * P], g2[:, icsl],
                                 start=True, stop=True)
        ex = exp_.tile([P, JG * IC], F32R, tag="ex", name="ex")
        nc.scalar.activation(ex[:], sT[:], AF.Exp)
        return ex

    def emit_mm2(ic, grp, ex):
        if grp == 0:
            state[ic] = dict(
                oA=[op.tile([P, HST], F32, tag="oacc", name=f"oA{ib}")
                    for ib in range(NIB)],
                oB=[op.tile([P, HST], F32, tag="oacc", name=f"oB{ib}")
                    for ib in range(NIB)] if tiled else None,
            )
        oA = state[ic]["oA"]; oB = state[ic]["oB"]
        for jbl in range(JG):
            jb = grp * JG + jbl
            hsl = slice(jb * HST, (jb + 1) * HST)
            st = jb == 0
            sp = jb == NJB - 1
            for ib in range(NIB):
                exsl = slice(jbl * IC + ib * P, jbl * IC + (ib + 1) * P)
                if tiled:
                    nc.tensor.matmul(oA[ib][:], ex[0:L, exsl],
                                     h_aug[0:L, hsl],
                                     start=st, stop=sp, tile_position=(0, 0))
                    nc.tensor.matmul(oB[ib][:], ex[L:P, exsl],
                                     h_aug[L:P, hsl],
                                     start=st, stop=sp, tile_position=(L, 0))
                else:
                    nc.tensor.matmul(oA[ib][:], ex[:, exsl], h_aug[:, hsl],
                                     start=st, stop=sp)

    def emit_epilogue(ic):
        oA = state[ic]["oA"]; oB = state[ic]["oB"]
        oscs = []
        for ib in range(NIB):
            if tiled:
                # DVE may read at most one PSUM operand per instruction:
                # stage oB through SBUF, then add oA (PSUM) + staged (SBUF).
                obs = ep.tile([P, HST], F32, tag="obs", name="obs")
                nc.vector.tensor_copy(obs[:], oB[ib][:])
                om = ep.tile([P, HST], F32, tag="om", name="om")
                nc.vector.tensor_add(om[:], oA[ib][:], obs[:])
                rs = om[:, C1:C1 + 1]
                src = om[:, 0:C1]
            else:
                rs = oA[ib][:, C1:C1 + 1]
                src = oA[ib][:, 0:C1]
            rv = ep.tile([P, 1], F32, tag="rv", name="rv")
            nc.vector.reciprocal(rv[:], rs[:])
            osc = ep.tile([P, C1], F32, tag="osc", name="osc")
            nc.vector.tensor_scalar(osc[:], src[:], rv[:], gm[:],
                                    op0=MUL, op1=MUL)
            oscs.append(osc)
        ost = [outp.tile([P, IC], F32, tag=f"ost{cb}", name=f"ost{cb}")
               for cb in range(2)]
        for ib in range(NIB):
            for cb in range(2):
                oT = ep.tile([P, P], BF16, tag="oTb", name="oT")
                nc.sync.dma_start_transpose(oT[:],
                                            oscs[ib][:, cb * P:(cb + 1) * P])
                nc.vector.tensor_add(
                    ost[cb][:, ib * P:(ib + 1) * P], oT[:],
                    q_sb[cb][:, ic * IC + ib * P:ic * IC + (ib + 1) * P])
        for cb in range(2):
            nc.sync.dma_start(o_out[cb * P:(cb + 1) * P, ic * IC:(ic + 1) * IC],
                              ost[cb][:])
        del state[ic]

    # Pipelined emission: mm1 runs one group ahead of mm2; the epilogue
    # (which puts transposes on the PE) is delayed one further group so the
    # exp stream never starves at an i-chunk boundary.
    groups = [(ic, grp) for ic in range(NIC) for grp in range(NGRP)]
    pending = None
    epi_due = None
    for (ic, grp) in groups:
        ex = emit_mm1(ic, grp)
        if epi_due is not None:
            emit_epilogue(epi_due)
            epi_due = None
        if pending is not None:
            pic, pgrp, pex = pending
            emit_mm2(pic, pgrp, pex)
            if pgrp == NGRP - 1:
                epi_due = pic
        pending = (ic, grp, ex)
    if epi_due is not None:
        emit_epilogue(epi_due)
    pic, pgrp, pex = pending
    emit_mm2(pic, pgrp, pex)
    emit_epilogue(pic)


def _preprocess(inputs, tiled=TILED):
    """Fold conv bias + BN into effective weights/biases, per-core input maps."""
    f32 = np.float32
    q = np.ascontiguousarray(inputs["q"], dtype=f32)[..., 0]   # [B, 256, N]
    k = np.ascontiguousarray(inputs["k"], dtype=f32)[..., 0]   # [B, 128, N]

    def fold(W, b, scale, bias, mean, var):
        inv = (np.asarray(scale, f32) /
               np.sqrt(np.asarray(var, f32) + f32(EPS))).astype(f32)
        W_eff = (inv[:, None] * np.asarray(W, f32)).astype(f32)
        delta = ((np.asarray(b, f32) - np.asarray(mean, f32)) * inv
                 + np.asarray(bias, f32)).astype(f32)
        return W_eff, delta

    Wf_e, d_f = fold(inputs["Wf"], inputs["bf"], inputs["fs"], inputs["fb"],
                     inputs["fm"], inputs["fv"])
    Wg_e, d_g = fold(inputs["Wg"], inputs["bg"], inputs["gs"], inputs["gb"],
                     inputs["gm"], inputs["gv"])
    Wh_e, d_h = fold(inputs["Wh"], inputs["bh"], inputs["hs"], inputs["hb"],
                     inputs["hm"], inputs["hv"])

    gamma = f32(np.asarray(inputs["gamma"], f32).reshape(-1)[0])
    if tiled:
        wf2 = np.concatenate([Wf_e.T, Wf_e.T], axis=1)        # [256, 128]
        wg2 = np.concatenate([Wg_e.T, Wg_e.T], axis=1)        # [128, 128]
        df2 = np.concatenate([d_f, d_f]).reshape(P, 1)
        dg2 = np.concatenate([d_g, d_g]).reshape(P, 1)
    else:
        # upper half must be exactly zero after gelu: zero weights and a
        # large negative bias (gelu(-30) underflows to 0)
        wf2 = np.concatenate([Wf_e.T, np.zeros_like(Wf_e.T)], axis=1)
        wg2 = np.concatenate([Wg_e.T, np.zeros_like(Wg_e.T)], axis=1)
        df2 = np.concatenate([d_f, np.full(L, -30.0, f32)]).reshape(P, 1)
        dg2 = np.concatenate([d_g, np.full(L, -30.0, f32)]).reshape(P, 1)

    ones2 = np.zeros((P, 2 * NJB), f32)
    ones2[:, 0::2] = 1.0

    shared = {
        "wf2T": _round_tf32(wf2),
        "wg2T": _round_tf32(wg2),
        "whT": _round_tf32(Wh_e.T),                           # [128, 256]
        "df2": df2,
        "dg2": dg2,
        "dht4": np.tile(np.broadcast_to(d_h, (P, C1)), (1, 4)).copy(),
        "gmb": np.full((P, 1), gamma, f32),
        "ones2": ones2,
    }
    in_maps = []
    for b_ in range(B):
        m = dict(shared)
        m["q2r"] = _round_tf32(q[b_])
        m["k2r"] = _round_tf32(k[b_])
        in_maps.append(m)
    return in_maps


def _get_nc():
    global _BUILT
    if _BUILT is None:
        _BUILT = _build()
    return _BUILT


def kernel(**inputs):
    nc = _get_nc()
    in_maps = _preprocess(inputs)
    res = run_bass_kernel_spmd(nc, in_maps, core_ids=list(range(B)))
    out = np.stack([res.results[i]["o_out"] for i in range(B)])
    return out[..., None].astype(np.float32)


if __name__ == "__main__":
    rng = np.random.default_rng(0)
    fake = {
        "q": rng.standard_normal((B, C1, N, 1), dtype=np.float32),
        "k": rng.standard_normal((B, C2, N, 1), dtype=np.float32),
        "Wf": rng.standard_normal((L, C1), dtype=np.float32) * 0.06,
        "bf": rng.standard_normal(L, dtype=np.float32) * 0.01,
        "fs": rng.random(L, dtype=np.float32) + 0.5,
        "fb": rng.standard_normal(L, dtype=np.float32) * 0.1,
        "fm": rng.standard_normal(L, dtype=np.float32) * 0.1,
        "fv": rng.random(L, dtype=np.float32) + 0.5,
        "Wg": rng.standard_normal((L, C2), dtype=np.float32) * 0.09,
        "bg": rng.standard_normal(L, dtype=np.float32) * 0.01,
        "gs": rng.random(L, dtype=np.float32) + 0.5,
        "gb": rng.standard_normal(L, dtype=np.float32) * 0.1,
        "gm": rng.standard_normal(L, dtype=np.float32) * 0.1,
        "gv": rng.random(L, dtype=np.float32) + 0.5,
        "Wh": rng.standard_normal((C1, C2), dtype=np.float32) * 0.09,
        "bh": rng.standard_normal(C1, dtype=np.float32) * 0.01,
        "hs": rng.random(C1, dtype=np.float32) + 0.5,
        "hb": rng.standard_normal(C1, dtype=np.float32) * 0.1,
        "hm": rng.standard_normal(C1, dtype=np.float32) * 0.1,
        "hv": rng.random(C1, dtype=np.float32) + 0.5,
        "gamma": np.array([-1.1], dtype=np.float32),
    }
    out = kernel(**fake)
    print("out", out.shape, out.dtype, float(np.abs(out).max()))
